# revision 9
# baseline (speedup 1.0000x reference)
"""Trainium2 kernel for nn_Loss_26886495273741 (retrieval_knn).

reference:
    dots = feature @ feature.T          # [n, n], n=16384, d=256
    dots[diag] = -1
    I = argmax(dots, axis=1)
    loss = -mean(log(n * ||feature - feature[I] + 1e-6||_2))

Strategy (8 NeuronCores, SPMD, no collectives):
  * Rows are sharded: core c owns rows [c*2048, (c+1)*2048).
  * Host passes F^T in fp8 twice: full ("ft", identical on all cores,
    the "all-gather" done by host replication) and the core's own row
    block ("at").
  * Device, per 128-row tile: fp8 DoubleRow matmuls fill 16 two-bank
    PSUM units [128, 1024] of fp32 dots (covering the 16384 columns).
    PSUM can only be read by the ACT and DVE engines (~1 col/cycle
    each), so the drain is the bottleneck; the 16 units are split
    ~evenly between the two engines (parity-alternating 7/9 and 8/8 so
    the average balances their speeds):
      - DVE max-absorbs its units into two independent bf16
        accumulators (two chains so the per-op semaphore round-trip of
        a single read-modify-write chain is hidden),
      - ACT copy-casts its units to bf16 staging tiles which are
        DMA-shipped to the host raw (the DMA engines are far from
        saturated, and folding them on-device would cost drain
        throughput).
    PSUM cycles as 4 two-bank buffers so up to 4 drains are in flight,
    and unit roles alternate engines so the buffer ring never
    serializes on one engine.  No on-device argmax at all.
  * Host maxes the shipped tiles (position-class maxima, class =
    col mod 1024), picks the top-7 classes per row (plus the
    diagonal's class), and evaluates the 16 candidate columns of each
    selected class in exact fp32 to recover the true argmax, then
    computes the reference loss formula.

The final loss is insensitive to near-tie argmax flips (each row
contributes 1/16384 of a log-term).
"""

import os
import sys

import numpy as np

# The axon PJRT plugin must be selectable: if a harness pinned
# JAX_PLATFORMS=cpu (common for running jax references), the device run
# would see no NeuronCores.  Prepending axon is a no-op when unset.
_jp = os.environ.get("JAX_PLATFORMS")
if _jp is not None and "axon" not in _jp:
    os.environ["JAX_PLATFORMS"] = "axon," + _jp

try:
    import concourse.bass as bass  # noqa: F401
except ImportError:  # grading env runs from a bare directory
    sys.path.insert(0, "/opt/trn_rl_repo")

import concourse.bass as bass
import concourse.mybir as mybir
import concourse.tile as tile
from concourse import bacc
from concourse.bass_utils import run_bass_kernel_spmd

# Problem geometry (hardcoded per spec.json: feature [16384, 256] f32).
N = 16384
D = 256
N_CORES = 8
ROWS_PER_CORE = N // N_CORES  # 2048
P = 128  # SBUF partitions
ROW_TILES = ROWS_PER_CORE // P  # 16
KH = D // P  # 2 contraction halves

UNIT = 1024  # drain unit width == 2 PSUM banks == matmul free dim
N_UNITS = N // UNIT  # 16 units per 128-row tile
W = UNIT  # position classes; host recovers N//W candidate cols per class
MM_WIDTH = 512  # matmul free dim (ISA max 512 per PSUM bank)

N_ACC = 2  # independent DVE accumulator chains

# Per-unit drain engine per row-tile parity: D = DVE, A = ACT.
# Even rows 7 D / 9 A, odd rows 8 D / 8 A (ACT is 1.25x faster per col).
PAT_EVEN = "ADADAADADADADADA"
PAT_ODD = "ADADADADADADADAD"
NV_EVEN = PAT_EVEN.count("A")  # 9
NV_ODD = PAT_ODD.count("A")  # 8
NV = max(NV_EVEN, NV_ODD)

TOPK = 7  # classes the host refines per row (plus the diagonal class)

EPS = 1e-6

_BF16 = mybir.dt.bfloat16
_F32 = mybir.dt.float32
_FP8 = mybir.dt.float8e4
_FP8_NP = mybir.dt.np(_FP8)

SHIP = tuple(f"md{i}" for i in range(N_ACC)) + tuple(f"mv{i}" for i in range(NV))


def build_nc(mm_width=MM_WIDTH):
    nc = bacc.Bacc("TRN2", target_bir_lowering=False, debug=False)

    # layout [P, KH, cols]: partition = k % 128, then k-half, then column
    ft_dram = nc.dram_tensor("ft", [P, KH, N], _FP8, kind="ExternalInput")
    at_dram = nc.dram_tensor("at", [P, KH, ROWS_PER_CORE], _FP8, kind="ExternalInput")
    # accumulators ship as bf16; ACT staging tiles ship as fp8 (the class
    # margin of the true argmax is ~4 sigma above fp8 quantization noise,
    # and it halves the host-bound DMA volume)
    outs = {
        name: nc.dram_tensor(
            name,
            [ROWS_PER_CORE, W],
            _BF16 if name.startswith("md") else _FP8,
            kind="ExternalOutput",
        )
        for name in SHIP
    }

    with tile.TileContext(nc) as tc:
        with (
            tc.tile_pool(name="ft_pool", bufs=1) as ft_pool,
            tc.tile_pool(name="at_pool", bufs=1) as at_pool,
            tc.tile_pool(name="acc_pool", bufs=3) as acc_pool,
            tc.tile_pool(name="s_pool", bufs=3) as s_pool,
            tc.tile_pool(name="psum", bufs=4, space="PSUM") as psum_pool,
        ):
            # Resident operands: F^T [128, 2, 16384] and the core's own
            # row block A^T [128, 2, 2048] (k-halves on the middle axis).
            at_sb = at_pool.tile([P, KH, ROWS_PER_CORE], _FP8, tag="at")
            ft_sb = ft_pool.tile([P, KH, N], _FP8, tag="ft")
            # load order: just what row-tile 0 unit 0 needs first, so the
            # compute pipeline fills as early as possible
            nc.sync.dma_start(at_sb[:, :, 0:128], at_dram[:, :, 0:128])
            nc.sync.dma_start(ft_sb[:, :, 0:1024], ft_dram[:, :, 0:1024])
            nc.sync.dma_start(at_sb[:, :, 128:], at_dram[:, :, 128:])
            for j in range(1024, N, 1024):
                nc.sync.dma_start(
                    ft_sb[:, :, j : j + 1024], ft_dram[:, :, j : j + 1024]
                )

            for r in range(ROW_TILES):
                pat = PAT_EVEN if r % 2 == 0 else PAT_ODD
                accs = [
                    acc_pool.tile([P, W], _BF16, tag=f"accD{i}", name=f"accD{i}_{r}")
                    for i in range(N_ACC)
                ]
                seeded = [False] * N_ACC
                vtiles = {}
                di = 0
                vi = 0
                for u, role in enumerate(pat):
                    ps = psum_pool.tile([P, UNIT], _F32, tag="ps")
                    for k in range(UNIT // mm_width):
                        c0 = u * UNIT + k * mm_width
                        nc.tensor.matmul(
                            ps[:, k * mm_width : (k + 1) * mm_width],
                            at_sb[:, :, r * P : (r + 1) * P],
                            ft_sb[:, :, c0 : c0 + mm_width],
                            start=True,
                            stop=True,
                            perf_mode=mybir.MatmulPerfMode.DoubleRow,
                        )
                    if role == "D":
                        a = accs[di % N_ACC]
                        if not seeded[di % N_ACC]:
                            nc.vector.tensor_copy(a[:], ps[:])
                            seeded[di % N_ACC] = True
                        else:
                            nc.vector.tensor_tensor(
                                a[:], ps[:], a[:], mybir.AluOpType.max
                            )
                        di += 1
                    else:
                        s = s_pool.tile([P, UNIT], _FP8, tag=f"v{vi}", name=f"v{vi}_{r}")
                        nc.scalar.copy(s[:], ps[:])
                        vtiles[f"v{vi}"] = s
                        vi += 1

                for i in range(N_ACC):
                    nc.sync.dma_start(outs[f"md{i}"][r * P : (r + 1) * P, :], accs[i][:])
                for vn, s in vtiles.items():
                    nc.sync.dma_start(outs["m" + vn][r * P : (r + 1) * P, :], s[:])

    nc.compile()
    return nc


_NC_CACHE = {}


def _get_nc():
    if "nc" not in _NC_CACHE:
        _NC_CACHE["nc"] = build_nc()
    return _NC_CACHE["nc"]


def make_inputs(feature: np.ndarray):
    """Host-side shard prep: F^T in [P, KH, cols] layout, quantized."""
    # ft[p, kh, j] = feature[j, kh*P + p]
    ft = np.ascontiguousarray(
        feature.T.reshape(KH, P, N).transpose(1, 0, 2)
    ).astype(_FP8_NP)
    in_maps = []
    for c in range(N_CORES):
        at = np.ascontiguousarray(
            ft[:, :, c * ROWS_PER_CORE : (c + 1) * ROWS_PER_CORE]
        )
        in_maps.append({"ft": ft, "at": at})
    return in_maps


def run_device(feature: np.ndarray, trace: bool = False):
    """Run the SPMD kernel; returns (vals [N, W] f32 class maxima, res)."""
    nc = _get_nc()
    in_maps = make_inputs(feature)
    res = run_bass_kernel_spmd(nc, in_maps, core_ids=list(range(N_CORES)), trace=trace)
    # mv tensors written only by even row-tiles (index >= NV_ODD) must be
    # masked for odd row-tiles (their DRAM is uninitialized there)
    row_tile_even = (np.arange(ROWS_PER_CORE) // P) % 2 == 0
    per_core = []
    for r in res.results:
        vals = r["md0"].astype(np.float32)
        for i in range(1, N_ACC):
            vals = np.maximum(vals, r[f"md{i}"].astype(np.float32))
        for i in range(NV):
            mv = r[f"mv{i}"].astype(np.float32)
            if i >= NV_ODD:
                mv = np.where(row_tile_even[:, None], mv, -np.inf)
            vals = np.maximum(vals, mv)
        per_core.append(vals)
    return np.concatenate(per_core), res


def recover_loss(feature: np.ndarray, vals: np.ndarray) -> np.float32:
    """Exact argmax recovery + reference loss formula on host.

    ``vals[i, c]`` is the device's (fp8-matmul, bf16-cast) max of
    ``dots[i, j]`` over columns j = c (mod W).  The top TOPK classes per
    row (plus the row's own diagonal class, which covers the case where
    the best neighbour hides under the self-dot) are evaluated in exact
    fp32.  Rows are processed grouped by class so candidate dot products
    are real GEMMs.
    """
    n = feature.shape[0]
    B = n // W  # candidate columns per class
    feat = np.ascontiguousarray(feature, dtype=np.float32)
    rows = np.arange(n)
    # top-TOPK classes per row by device value
    t_cls = np.argpartition(-vals, TOPK, axis=1)[:, :TOPK].astype(np.int64)

    best_val = np.full(n, -np.inf, dtype=np.float32)
    best_col = np.zeros(n, dtype=np.int64)

    def consider(row_idx: np.ndarray, t: int):
        """Evaluate class-t candidate columns for the given rows."""
        cols = t + W * np.arange(B)  # [B]
        cd = feat[row_idx] @ feat[cols].T  # [len(rows), B] exact fp32
        self_b = np.where(row_idx % W == t, row_idx // W, -1)
        k = np.arange(len(row_idx))
        has_self = self_b >= 0
        cd[k[has_self], self_b[has_self]] = -np.inf
        b = np.argmax(cd, axis=1)
        v = cd[k, b]
        c = cols[b]
        upd = (v > best_val[row_idx]) | (
            (v == best_val[row_idx]) & (c < best_col[row_idx])
        )
        ri = row_idx[upd]
        best_val[ri] = v[upd]
        best_col[ri] = c[upd]

    for k in range(t_cls.shape[1]):
        col = t_cls[:, k]
        order = np.argsort(col, kind="stable")
        bounds = np.searchsorted(col[order], np.arange(W + 1))
        for t in range(W):
            grp = order[bounds[t] : bounds[t + 1]]
            if len(grp):
                consider(grp, t)
    for t in range(W):
        consider(rows[t::W], t)  # rows whose diagonal falls in class t

    I = best_col
    diff = feat - feat[I] + EPS
    dist = np.sqrt((diff * diff).sum(axis=1))
    loss = -np.mean(np.log(n * dist))
    return np.float32(loss)


def kernel(feature: np.ndarray) -> np.ndarray:
    feature = np.asarray(feature, dtype=np.float32)
    try:
        vals, _res = run_device(feature)
    except Exception:
        # one retry for transient device/tunnel hiccups
        _NC_CACHE.clear()
        vals, _res = run_device(feature)
    return np.asarray(recover_loss(feature, vals), dtype=np.float32)


if __name__ == "__main__":
    rng = np.random.default_rng(0)
    feature = rng.standard_normal((N, D), dtype=np.float32)
    print("loss:", kernel(feature))


# revision 12
# speedup vs baseline: 1.0013x; 1.0013x over previous
"""Trainium2 kernel for nn_Loss_26886495273741 (retrieval_knn).

reference:
    dots = feature @ feature.T          # [n, n], n=16384, d=256
    dots[diag] = -1
    I = argmax(dots, axis=1)
    loss = -mean(log(n * ||feature - feature[I] + 1e-6||_2))

Strategy (8 NeuronCores, SPMD, no collectives):
  * Rows are sharded: core c owns rows [c*2048, (c+1)*2048).
  * Host passes F^T in fp8 twice: full ("ft", identical on all cores,
    the "all-gather" done by host replication) and the core's own row
    block ("at").
  * Device, per 128-row tile: fp8 DoubleRow matmuls fill 16 two-bank
    PSUM units [128, 1024] of fp32 dots (covering the 16384 columns).
    PSUM can only be read by the ACT and DVE engines (~1 col/cycle
    each), so the drain is the bottleneck; the 16 units are split
    ~evenly between the two engines (parity-alternating 7/9 and 8/8 so
    the average balances their speeds):
      - DVE max-absorbs its units into two independent bf16
        accumulators (two chains so the per-op semaphore round-trip of
        a single read-modify-write chain is hidden),
      - ACT copy-casts its units to bf16 staging tiles which are
        DMA-shipped to the host raw (the DMA engines are far from
        saturated, and folding them on-device would cost drain
        throughput).
    PSUM cycles as 4 two-bank buffers so up to 4 drains are in flight,
    and unit roles alternate engines so the buffer ring never
    serializes on one engine.  No on-device argmax at all.
  * Host maxes the shipped tiles (position-class maxima, class =
    col mod 1024), picks the top-7 classes per row (plus the
    diagonal's class), and evaluates the 16 candidate columns of each
    selected class in exact fp32 to recover the true argmax, then
    computes the reference loss formula.

The final loss is insensitive to near-tie argmax flips (each row
contributes 1/16384 of a log-term).
"""

import os
import sys

import numpy as np

# The axon PJRT plugin must be selectable: if a harness pinned
# JAX_PLATFORMS=cpu (common for running jax references), the device run
# would see no NeuronCores.  Prepending axon is a no-op when unset.
_jp = os.environ.get("JAX_PLATFORMS")
if _jp is not None and "axon" not in _jp:
    os.environ["JAX_PLATFORMS"] = "axon," + _jp

try:
    import concourse.bass as bass  # noqa: F401
except ImportError:  # grading env runs from a bare directory
    sys.path.insert(0, "/opt/trn_rl_repo")

import concourse.bass as bass
import concourse.mybir as mybir
import concourse.tile as tile
from concourse import bacc
from concourse.bass_utils import run_bass_kernel_spmd

# Problem geometry (hardcoded per spec.json: feature [16384, 256] f32).
N = 16384
D = 256
N_CORES = 8
ROWS_PER_CORE = N // N_CORES  # 2048
P = 128  # SBUF partitions
ROW_TILES = ROWS_PER_CORE // P  # 16
KH = D // P  # 2 contraction halves

UNIT = 1024  # drain unit width == 2 PSUM banks == matmul free dim
N_UNITS = N // UNIT  # 16 units per 128-row tile
W = UNIT  # position classes; host recovers N//W candidate cols per class
MM_WIDTH = 512  # matmul free dim (ISA max 512 per PSUM bank)

N_ACC = 2  # independent DVE accumulator chains

# Per-unit drain engine per row-tile parity: D = DVE, A = ACT.
# Even rows 7 D / 9 A, odd rows 8 D / 8 A (ACT is 1.25x faster per col).
PAT_EVEN = "ADADAADADADADADA"
PAT_ODD = "ADADADADADADADAD"
# the final row-tile has no successor work to overlap its ring; a
# front-loaded ACT-heavy pattern empirically shortens the tail
PAT_LAST = "AADDAADADADAADAD"
PATS = [PAT_EVEN if r % 2 == 0 else PAT_ODD for r in range(ROW_TILES)]
PATS[ROW_TILES - 1] = PAT_LAST
NV = max(p.count("A") for p in PATS)

TOPK = 7  # classes the host refines per row (plus the diagonal class)

EPS = 1e-6

_BF16 = mybir.dt.bfloat16
_F32 = mybir.dt.float32
_FP8 = mybir.dt.float8e4
_FP8_NP = mybir.dt.np(_FP8)

SHIP = tuple(f"md{i}" for i in range(N_ACC)) + tuple(f"mv{i}" for i in range(NV))


def build_nc(mm_width=MM_WIDTH):
    nc = bacc.Bacc("TRN2", target_bir_lowering=False, debug=False)

    # layout [P, KH, cols]: partition = k % 128, then k-half, then column
    ft_dram = nc.dram_tensor("ft", [P, KH, N], _FP8, kind="ExternalInput")
    at_dram = nc.dram_tensor("at", [P, KH, ROWS_PER_CORE], _FP8, kind="ExternalInput")
    # accumulators ship as bf16; ACT staging tiles ship as fp8 (the class
    # margin of the true argmax is ~4 sigma above fp8 quantization noise,
    # and it halves the host-bound DMA volume)
    outs = {
        name: nc.dram_tensor(
            name,
            [ROWS_PER_CORE, W],
            _BF16 if name.startswith("md") else _FP8,
            kind="ExternalOutput",
        )
        for name in SHIP
    }

    with tile.TileContext(nc) as tc:
        with (
            tc.tile_pool(name="ft_pool", bufs=1) as ft_pool,
            tc.tile_pool(name="at_pool", bufs=1) as at_pool,
            tc.tile_pool(name="acc_pool", bufs=3) as acc_pool,
            tc.tile_pool(name="s_pool", bufs=3) as s_pool,
            tc.tile_pool(name="psum", bufs=4, space="PSUM") as psum_pool,
        ):
            # Resident operands: F^T [128, 2, 16384] and the core's own
            # row block A^T [128, 2, 2048] (k-halves on the middle axis).
            at_sb = at_pool.tile([P, KH, ROWS_PER_CORE], _FP8, tag="at")
            ft_sb = ft_pool.tile([P, KH, N], _FP8, tag="ft")
            # load order: just what row-tile 0 unit 0 needs first, so the
            # compute pipeline fills as early as possible
            nc.sync.dma_start(at_sb[:, :, 0:128], at_dram[:, :, 0:128])
            nc.sync.dma_start(ft_sb[:, :, 0:1024], ft_dram[:, :, 0:1024])
            nc.sync.dma_start(at_sb[:, :, 128:], at_dram[:, :, 128:])
            for j in range(1024, N, 1024):
                nc.sync.dma_start(
                    ft_sb[:, :, j : j + 1024], ft_dram[:, :, j : j + 1024]
                )

            for r in range(ROW_TILES):
                pat = PATS[r]
                accs = [
                    acc_pool.tile([P, W], _BF16, tag=f"accD{i}", name=f"accD{i}_{r}")
                    for i in range(N_ACC)
                ]
                seeded = [False] * N_ACC
                vtiles = {}
                di = 0
                vi = 0
                for u, role in enumerate(pat):
                    ps = psum_pool.tile([P, UNIT], _F32, tag="ps")
                    for k in range(UNIT // mm_width):
                        c0 = u * UNIT + k * mm_width
                        nc.tensor.matmul(
                            ps[:, k * mm_width : (k + 1) * mm_width],
                            at_sb[:, :, r * P : (r + 1) * P],
                            ft_sb[:, :, c0 : c0 + mm_width],
                            start=True,
                            stop=True,
                            perf_mode=mybir.MatmulPerfMode.DoubleRow,
                        )
                    if role == "D":
                        a = accs[di % N_ACC]
                        if not seeded[di % N_ACC]:
                            nc.vector.tensor_copy(a[:], ps[:])
                            seeded[di % N_ACC] = True
                        else:
                            nc.vector.tensor_tensor(
                                a[:], ps[:], a[:], mybir.AluOpType.max
                            )
                        di += 1
                    else:
                        s = s_pool.tile([P, UNIT], _FP8, tag=f"v{vi}", name=f"v{vi}_{r}")
                        nc.scalar.copy(s[:], ps[:])
                        vtiles[f"v{vi}"] = s
                        vi += 1

                for i in range(N_ACC):
                    nc.sync.dma_start(outs[f"md{i}"][r * P : (r + 1) * P, :], accs[i][:])
                for vn, s in vtiles.items():
                    nc.sync.dma_start(outs["m" + vn][r * P : (r + 1) * P, :], s[:])

    nc.compile()
    return nc


_NC_CACHE = {}


def _get_nc():
    if "nc" not in _NC_CACHE:
        _NC_CACHE["nc"] = build_nc()
    return _NC_CACHE["nc"]


def make_inputs(feature: np.ndarray):
    """Host-side shard prep: F^T in [P, KH, cols] layout, quantized."""
    # ft[p, kh, j] = feature[j, kh*P + p]
    ft = np.ascontiguousarray(
        feature.T.reshape(KH, P, N).transpose(1, 0, 2)
    ).astype(_FP8_NP)
    in_maps = []
    for c in range(N_CORES):
        at = np.ascontiguousarray(
            ft[:, :, c * ROWS_PER_CORE : (c + 1) * ROWS_PER_CORE]
        )
        in_maps.append({"ft": ft, "at": at})
    return in_maps


def run_device(feature: np.ndarray, trace: bool = False):
    """Run the SPMD kernel; returns (vals [N, W] f32 class maxima, res)."""
    nc = _get_nc()
    in_maps = make_inputs(feature)
    res = run_bass_kernel_spmd(nc, in_maps, core_ids=list(range(N_CORES)), trace=trace)
    # mv tensor i is only written by row-tiles whose pattern has > i ACT
    # units; mask the rest (their DRAM is uninitialized there)
    row_tile_of = np.arange(ROWS_PER_CORE) // P
    per_core = []
    for r in res.results:
        vals = r["md0"].astype(np.float32)
        for i in range(1, N_ACC):
            vals = np.maximum(vals, r[f"md{i}"].astype(np.float32))
        for i in range(NV):
            mv = r[f"mv{i}"].astype(np.float32)
            valid = np.array([p.count("A") > i for p in PATS])[row_tile_of]
            mv = np.where(valid[:, None], mv, -np.inf)
            vals = np.maximum(vals, mv)
        per_core.append(vals)
    return np.concatenate(per_core), res


def recover_loss(feature: np.ndarray, vals: np.ndarray) -> np.float32:
    """Exact argmax recovery + reference loss formula on host.

    ``vals[i, c]`` is the device's (fp8-matmul, bf16-cast) max of
    ``dots[i, j]`` over columns j = c (mod W).  The top TOPK classes per
    row (plus the row's own diagonal class, which covers the case where
    the best neighbour hides under the self-dot) are evaluated in exact
    fp32.  Rows are processed grouped by class so candidate dot products
    are real GEMMs.
    """
    n = feature.shape[0]
    B = n // W  # candidate columns per class
    feat = np.ascontiguousarray(feature, dtype=np.float32)
    rows = np.arange(n)
    # top-TOPK classes per row by device value
    t_cls = np.argpartition(-vals, TOPK, axis=1)[:, :TOPK].astype(np.int64)

    best_val = np.full(n, -np.inf, dtype=np.float32)
    best_col = np.zeros(n, dtype=np.int64)

    def consider(row_idx: np.ndarray, t: int):
        """Evaluate class-t candidate columns for the given rows."""
        cols = t + W * np.arange(B)  # [B]
        cd = feat[row_idx] @ feat[cols].T  # [len(rows), B] exact fp32
        self_b = np.where(row_idx % W == t, row_idx // W, -1)
        k = np.arange(len(row_idx))
        has_self = self_b >= 0
        cd[k[has_self], self_b[has_self]] = -np.inf
        b = np.argmax(cd, axis=1)
        v = cd[k, b]
        c = cols[b]
        upd = (v > best_val[row_idx]) | (
            (v == best_val[row_idx]) & (c < best_col[row_idx])
        )
        ri = row_idx[upd]
        best_val[ri] = v[upd]
        best_col[ri] = c[upd]

    for k in range(t_cls.shape[1]):
        col = t_cls[:, k]
        order = np.argsort(col, kind="stable")
        bounds = np.searchsorted(col[order], np.arange(W + 1))
        for t in range(W):
            grp = order[bounds[t] : bounds[t + 1]]
            if len(grp):
                consider(grp, t)
    for t in range(W):
        consider(rows[t::W], t)  # rows whose diagonal falls in class t

    I = best_col
    diff = feat - feat[I] + EPS
    dist = np.sqrt((diff * diff).sum(axis=1))
    loss = -np.mean(np.log(n * dist))
    return np.float32(loss)


def kernel(feature: np.ndarray) -> np.ndarray:
    feature = np.asarray(feature, dtype=np.float32)
    try:
        vals, _res = run_device(feature)
    except Exception:
        # one retry for transient device/tunnel hiccups
        _NC_CACHE.clear()
        vals, _res = run_device(feature)
    return np.asarray(recover_loss(feature, vals), dtype=np.float32)


if __name__ == "__main__":
    rng = np.random.default_rng(0)
    feature = rng.standard_normal((N, D), dtype=np.float32)
    print("loss:", kernel(feature))
